# revision 12
# baseline (speedup 1.0000x reference)
"""DualMambaBlock Trainium2 kernel: 8-core SPMD Bass/Tile implementation.

Reference computes (B=4, L=256, C=32, D=128, DI=256, DS=16, DC=4, DR=8):
  T_out = temporal mamba over L (batch B*C)     -> [B,L,C,D]
  C_out = channel mamba over C (batch B*L) on gated x -> [B,L,C,D]
  gate g[b,c] = sigmoid(||mean_l(x) @ g_w + g_b||^2 / 8)

Sharding: core k handles b = k//2;  temporal: c in [16*(k%2), +16);
channel: l in [128*(k%2), +128).  Gate mean needs full L so each core also
reads the sibling half of x[b].

Device layout: everything is [d_partition(128) x tokens] ("transposed"),
tokens are (seq-major, t contiguous).  Host pre-transposes inputs and
post-transposes outputs, so all DMAs are contiguous.

v2: conv folded into tap-scaled in-proj matmuls (TensorE) with shifted-x
tiles; brep/crep broadcasts shared across q; activation-table thrash
removed (batched Exp then Ln, dt2 via copy+poison); y2 finalize uses
2x-mode ops.
"""
import numpy as np
import ml_dtypes

import concourse.bass as bass
import concourse.bacc as bacc
import concourse.tile as tile
import concourse.mybir as mybir
from concourse.bass_utils import run_bass_kernel_spmd

F32 = mybir.dt.float32
BF16 = mybir.dt.bfloat16
AF = mybir.ActivationFunctionType
ALU = mybir.AluOpType
BF = ml_dtypes.bfloat16

B, L, C, D = 4, 256, 32, 128
DI, DS, DC, DR = 256, 16, 4, 8
ST = 4096            # tokens per core per mamba
NCH = 4              # chunks over ST (F=1024 so 2 PSUM y-accumulators fit)
F = ST // NCH        # tokens per chunk
POISON = 40.0        # dt value whose exp(-k*dt) underflows to 0 for k>=1

_CACHE = {}
TRACE = False


def _ap3(t, p_ap, d0, d1):
    """view [128, d0(step0), d1] of a [128, d1] tile (free broadcast)."""
    return bass.AP(p_ap.tensor, p_ap.offset, [p_ap.ap[0], [0, d0], p_ap.ap[1]])


def build_program():
    nc = bacc.Bacc('TRN2', target_bir_lowering=False, debug=False, num_devices=8)

    def din(name, shape, dt=BF16):
        return nc.dram_tensor(name, shape, dt, kind='ExternalInput').ap()

    def dout(name, shape, dt=F32):
        return nc.dram_tensor(name, shape, dt, kind='ExternalOutput').ap()

    # per-core data
    xt = din('xt', [128, ST])          # temporal tokens (c-major, l contig)
    xc = din('xc', [128, ST])          # channel tokens own l-half (l-major, c contig)
    xo = din('xo', [128, ST])          # other l-half (for gate mean only)
    t_out = dout('t_out', [128, ST])
    c_out = dout('c_out', [128, ST])

    # weights (shared across cores); layouts chosen for direct DMA as lhsT
    w = {}
    for p in ('t', 'c'):
        # u-half in-proj pre-scaled by conv tap j (host): [D, 4*DI] col j*DI+di
        w[p + '_w_in_u'] = din(p + '_w_in_u', [D, DC * DI])
        w[p + '_w_in_z'] = din(p + '_w_in_z', [D, DI])       # lhsT [128, 256]
        w[p + '_dteff'] = din(p + '_dteff', [DI, DI])         # lhsT [K=256, 256]
        w[p + '_w_bc'] = din(p + '_w_bc', [DI, 2 * DS])       # lhsT [K=256, 32]
        w[p + '_w_out'] = din(p + '_w_out', [DI, D])          # lhsT [K=256, 128]
        w[p + '_conv_b'] = din(p + '_conv_b', [DI], F32)
        w[p + '_b_dt'] = din(p + '_b_dt', [DI], F32)
        w[p + '_d'] = din(p + '_d', [DI], F32)
        w[p + '_a'] = din(p + '_a', [DS], F32)                # -exp(a_log[0])
    w['g_w'] = din('g_w', [D, D // 2])                        # pre-scaled by 1/L
    w['g_b'] = din('g_b', [D // 2], F32)

    with tile.TileContext(nc) as tc:
        import contextlib
        with contextlib.ExitStack() as ctx:
            wp = ctx.enter_context(tc.tile_pool(name='wp', bufs=1))
            xp = ctx.enter_context(tc.tile_pool(name='xp', bufs=1))
            big = ctx.enter_context(tc.tile_pool(name='big', bufs=1))
            work = ctx.enter_context(tc.tile_pool(name='work', bufs=4))
            sm = ctx.enter_context(tc.tile_pool(name='sm', bufs=2))
            # 8 PSUM banks: mm_t(2) + mm_c(2) + y_ps0(2) + y_ps1(2)
            ps_a = ctx.enter_context(tc.tile_pool(name='ps_a', bufs=2, space='PSUM'))
            ps_y = ctx.enter_context(tc.tile_pool(name='ps_y', bufs=1, space='PSUM'))

            # ---- load weights ------------------------------------------------
            sb = {}
            for name, ap in w.items():
                shape = list(ap.shape)
                if len(shape) == 1 and shape[0] == DI:
                    # [256] vector -> [128, 2] tile; column q holds di-tile q
                    t = wp.tile([128, 2], ap.dtype, tag='w_' + name)
                    nc.sync.dma_start(
                        out=t[:],
                        in_=bass.AP(ap.tensor, ap.offset, [[1, 128], [128, 2]]))
                elif len(shape) == 1 and shape[0] == DS:
                    # [16] vector -> broadcast across 128 partitions
                    t = wp.tile([128, DS], ap.dtype, tag='w_' + name)
                    nc.sync.dma_start(
                        out=t[:],
                        in_=bass.AP(ap.tensor, ap.offset, [[0, 128], [1, DS]]))
                elif len(shape) == 1:
                    t = wp.tile([shape[0], 1], ap.dtype, tag='w_' + name)
                    nc.sync.dma_start(out=t[:], in_=ap[:, None])
                elif shape[0] > 128:
                    # split K=256 weights into two [128, N] tiles
                    t = []
                    for kq in range(2):
                        tt = wp.tile([128, shape[1]], ap.dtype,
                                     tag=f'w_{name}_{kq}', name=f'w_{name}_{kq}')
                        nc.sync.dma_start(out=tt[:],
                                          in_=ap[kq * 128:(kq + 1) * 128, :])
                        t.append(tt)
                else:
                    t = wp.tile(shape, ap.dtype, tag='w_' + name)
                    nc.sync.dma_start(out=t[:], in_=ap[:])
                sb[name] = t

            from concourse.masks import make_identity
            ones1 = wp.tile([1, 128], BF16, tag='ones1')
            nc.vector.memset(ones1[:], 1.0)
            ones64 = wp.tile([64, 1], BF16, tag='ones64')
            nc.vector.memset(ones64[:], 1.0)
            ident = wp.tile([128, 128], BF16, tag='ident')
            make_identity(nc, ident[:])

            # x tiles (xt chunked so chunk-0 work starts early)
            xt_s = xp.tile([128, ST], BF16, tag='xt')
            for ch in range(NCH):
                cc = slice(ch * F, (ch + 1) * F)
                nc.sync.dma_start(out=xt_s[:, cc], in_=xt[:, cc])
            xc_s = xp.tile([128, ST], BF16, tag='xc')
            nc.sync.dma_start(out=xc_s[:], in_=xc[:])
            xo_s = xp.tile([128, ST], BF16, tag='xo')
            nc.sync.dma_start(out=xo_s[:], in_=xo[:])

            # ---- gate --------------------------------------------------------
            # mean over l: view [d; c, l] of l-major tokens (col = l*32+c)
            m1 = sm.tile([128, C], F32, tag='m1')
            m2 = sm.tile([128, C], F32, tag='m2')
            nc.vector.reduce_sum(
                m1[:], bass.AP(xc_s[:].tensor, xc_s[:].offset,
                               [xc_s[:].ap[0], [1, C], [C, 128]]),
                axis=mybir.AxisListType.X)
            nc.vector.reduce_sum(
                m2[:], bass.AP(xo_s[:].tensor, xo_s[:].offset,
                               [xo_s[:].ap[0], [1, C], [C, 128]]),
                axis=mybir.AxisListType.X)
            msum = sm.tile([128, C], BF16, tag='msum')
            nc.vector.tensor_add(msum[:], m1[:], m2[:])
            node_ps = ps_a.tile([64, C], F32, tag='mm_c')
            nc.tensor.matmul(node_ps[:], sb['g_w'][:], msum[:], start=True, stop=True)
            node_sq = sm.tile([64, C], BF16, tag='node_sq')
            nc.scalar.activation(node_sq[:], node_ps[:], AF.Square,
                                 bias=sb['g_b'][:], scale=1.0)
            nrm_ps = ps_a.tile([1, C], F32, tag='mm_c')
            nc.tensor.matmul(nrm_ps[:], ones64[:], node_sq[:], start=True, stop=True)
            g_row = sm.tile([1, C], BF16, tag='g_row')
            nc.scalar.activation(g_row[:], nrm_ps[:], AF.Sigmoid, scale=0.125)
            grep_ps = ps_a.tile([128, C], F32, tag='mm_c')
            nc.tensor.matmul(grep_ps[:], ones1[:], g_row[:], start=True, stop=True)
            g_tile = sm.tile([128, C], BF16, tag='g_tile')
            nc.scalar.copy(g_tile[:], grep_ps[:])
            # xg = xc * g (broadcast over l via step-0); reuse xo slot
            xg_s = xp.tile([128, ST], BF16, tag='xo')
            nc.vector.tensor_mul(
                xg_s[:].rearrange('p (l c) -> p l c', c=C),
                xc_s[:].rearrange('p (l c) -> p l c', c=C),
                _ap3(g_tile, g_tile[:], L // 2, C))

            # ---- mamba body (staged, so c's phase A overlaps t's phase B) ----
            class Mamba:
                def __init__(self, pfx, xsrc, T, out_dram):
                    self.pfx, self.xsrc, self.T, self.out_dram = pfx, xsrc, T, out_dram
                    self.u = [big.tile([128, ST], BF16, tag=f'{pfx}_u{q}',
                                       name=f'{pfx}_u{q}') for q in range(2)]
                    self.dt2 = [big.tile([128, ST], BF16, tag=f'{pfx}_dt2{q}',
                                         name=f'{pfx}_dt2{q}') for q in range(2)]
                    self.bc_dram = nc.dram_tensor(f'{pfx}_bc_scratch',
                                                  [2 * DS, ST], BF16).ap()
                    self.zs_dram = nc.dram_tensor(f'{pfx}_zs_scratch',
                                                  [2 * 128, ST], BF16).ap()
                    self.mm = f'mm_{pfx}'
                    # diag(d) per q for the u*D skip accumulation
                    self.dd = []
                    for q in range(2):
                        dd = wp.tile([128, 128], BF16, tag=f'dd_{pfx}{q}')
                        nc.vector.tensor_scalar_mul(
                            dd[:], ident[:], sb[pfx + '_d'][:, q:q + 1])
                        self.dd.append(dd)

                def stage0(self):
                    """shifted copies of xsrc for conv taps (zero at seq starts)"""
                    T = self.T
                    self.xs = [big.tile([128, ST], BF16, tag=f'xs{s}',
                                        name=f'{self.pfx}_xs{s}') for s in range(3)]
                    self.et = [big.tile([128, ST], BF16, tag=f'et{q}',
                                        name=f'{self.pfx}_et{q}') for q in range(2)]
                    self.bc = big.tile([2 * DS, ST], BF16, tag='bc',
                                       name=f'{self.pfx}_bc')
                    for ch in range(NCH):
                        c0, c1 = ch * F, (ch + 1) * F
                        for s in range(3):
                            sh = s + 1
                            o0 = max(c0, sh)
                            nc.sync.dma_start(out=self.xs[s][:, o0:c1],
                                              in_=self.xsrc[:, o0 - sh:c1 - sh])
                            nc.gpsimd.memset(
                                self.xs[s][:, c0:c1].rearrange(
                                    'p (s t) -> p s t', t=T)[:, :, 0:sh], 0.0)

                def stageA(self, ch):
                    """in-proj (fused conv taps) + silu, dt-proj + softplus,
                    B/C projection — for one chunk"""
                    w_in_u = sb[self.pfx + '_w_in_u']
                    conv_b = sb[self.pfx + '_conv_b']
                    dteff = sb[self.pfx + '_dteff']
                    b_dt = sb[self.pfx + '_b_dt']
                    w_bc = sb[self.pfx + '_w_bc']
                    jblk = [slice(ch * F + j * 512, ch * F + (j + 1) * 512)
                            for j in range(F // 512)]
                    for q in range(2):
                        qs = slice(q * 128, (q + 1) * 128)
                        for cols in jblk:
                            up = ps_a.tile([128, 512], F32, tag=self.mm)
                            # tap 3 (no shift) from xsrc; taps 2,1,0 shifted
                            nc.tensor.matmul(up[:], w_in_u[:, 3 * DI:][:, qs],
                                             self.xsrc[:, cols], start=True, stop=False)
                            nc.tensor.matmul(up[:], w_in_u[:, 2 * DI:3 * DI][:, qs],
                                             self.xs[0][:, cols], start=False, stop=False)
                            nc.tensor.matmul(up[:], w_in_u[:, 1 * DI:2 * DI][:, qs],
                                             self.xs[1][:, cols], start=False, stop=False)
                            nc.tensor.matmul(up[:], w_in_u[:, 0 * DI:1 * DI][:, qs],
                                             self.xs[2][:, cols], start=False, stop=True)
                            nc.scalar.activation(self.u[q][:, cols], up[:], AF.Silu,
                                                 bias=conv_b[:, q:q + 1], scale=1.0)
                    for q in range(2):
                        for cols in jblk:
                            dp = ps_a.tile([128, 512], F32, tag=self.mm)
                            nc.tensor.matmul(dp[:], dteff[0][:, q * 128:(q + 1) * 128],
                                             self.u[0][:, cols], start=True, stop=False)
                            nc.tensor.matmul(dp[:], dteff[1][:, q * 128:(q + 1) * 128],
                                             self.u[1][:, cols], start=False, stop=True)
                            nc.scalar.activation(self.et[q][:, cols], dp[:], AF.Exp,
                                                 bias=b_dt[:, q:q + 1], scale=1.0)
                    ccols = slice(ch * F, (ch + 1) * F)
                    for q in range(2):
                        # softplus(x) = ln(exp(x) + 1)
                        nc.scalar.activation(self.dt2[q][:, ccols],
                                             self.et[q][:, ccols], AF.Ln, bias=1.0)
                    for cols in jblk:
                        bp = ps_a.tile([32, 512], F32, tag=self.mm)
                        nc.tensor.matmul(bp[:], w_bc[0][:], self.u[0][:, cols],
                                         start=True, stop=False)
                        nc.tensor.matmul(bp[:], w_bc[1][:], self.u[1][:, cols],
                                         start=False, stop=True)
                        nc.scalar.copy(self.bc[:, cols], bp[:])
                        nc.sync.dma_start(out=self.bc_dram[:, cols],
                                          in_=self.bc[:, cols])

                def stageZ(self):
                    """z-proj + silu -> DRAM, all chunks (one Silu table pass)"""
                    w_in_z = sb[self.pfx + '_w_in_z']
                    for q in range(2):
                        qs = slice(q * 128, (q + 1) * 128)
                        for j in range(ST // 512):
                            cols = slice(j * 512, (j + 1) * 512)
                            zp = ps_a.tile([128, 512], F32, tag=self.mm)
                            nc.tensor.matmul(zp[:], w_in_z[:, qs],
                                             self.xsrc[:, cols], start=True, stop=True)
                            zstage = work.tile([128, 512], BF16, tag='zstage',
                                               name='zstage', bufs=2)
                            nc.scalar.activation(zstage[:], zp[:], AF.Silu)
                            nc.sync.dma_start(
                                out=self.zs_dram[q * 128:(q + 1) * 128, cols],
                                in_=zstage[:])

                def stageB(self, ch, sprinkles=None):
                    """ds loop for one chunk + finalize + out-proj"""
                    T = self.T
                    w_out = sb[self.pfx + '_w_out']
                    a_vec = sb[self.pfx + '_a']
                    cols = slice(ch * F, (ch + 1) * F)
                    y_ps = [ps_y.tile([128, F], F32, tag=f'y_ps{q}',
                                      name=f'y_ps{q}') for q in range(2)]
                    dtu = []
                    for q in range(2):
                        # skip path: y_ps starts as diag(d) @ u
                        for j in range(F // 512):
                            o2 = slice(j * 512, (j + 1) * 512)
                            c2 = slice(ch * F + j * 512, ch * F + (j + 1) * 512)
                            nc.tensor.matmul(y_ps[q][:, o2], self.dd[q][:],
                                             self.u[q][:, c2],
                                             start=True, stop=False)
                        dtuq = work.tile([128, F], BF16, tag=f'dtu{q}',
                                         name=f'dtu{q}', bufs=2)
                        nc.vector.tensor_mul(dtuq[:], self.dt2[q][:, cols],
                                             self.u[q][:, cols])
                        dtu.append(dtuq)
                    for ds in range(DS):
                        if sprinkles and ds in sprinkles:
                            for fn in sprinkles[ds]:
                                fn()
                        brep = work.tile([128, F], BF16, tag='brep', name='brep', bufs=2)
                        crep = work.tile([128, F], BF16, tag='crep', name='crep', bufs=2)
                        nc.sync.dma_start(
                            out=brep[:],
                            in_=bass.AP(self.bc_dram.tensor, ds * ST + ch * F,
                                        [[0, 128], [1, F]]))
                        nc.sync.dma_start(
                            out=crep[:],
                            in_=bass.AP(self.bc_dram.tensor, (DS + ds) * ST + ch * F,
                                        [[0, 128], [1, F]]))
                        for q in range(2):
                            dA = work.tile([128, F], BF16, tag='dA', name='dA', bufs=4)
                            nc.scalar.activation(dA[:], self.dt2[q][:, cols], AF.Exp,
                                                 scale=a_vec[:, ds:ds + 1])
                            # scan reset at sequence starts
                            nc.gpsimd.memset(
                                dA[:].rearrange('p (s t) -> p s t', t=T)[:, :, 0:1],
                                0.0)
                            in1 = work.tile([128, F], BF16, tag='in1', name='in1', bufs=2)
                            nc.vector.tensor_mul(in1[:], dtu[q][:], brep[:])
                            h = work.tile([128, F], BF16, tag='h', name='h', bufs=2)
                            nc.vector.tensor_tensor_scan(
                                h[:], dA[:], in1[:], 0.0,
                                op0=ALU.mult, op1=ALU.add)
                            hc = work.tile([128, F], BF16, tag='hc', name='hc', bufs=2)
                            nc.vector.tensor_mul(hc[:], h[:], crep[:])
                            for j in range(F // 512):
                                o2 = slice(j * 512, (j + 1) * 512)
                                nc.tensor.matmul(y_ps[q][:, o2], ident[:], hc[:, o2],
                                                 start=False, stop=(ds == DS - 1))
                    y2 = []
                    for q in range(2):
                        # finalize: y = (y_scan + u*D) * silu(z)
                        ysb = work.tile([128, F], BF16, tag='ysb', name='ysb', bufs=2)
                        nc.scalar.copy(ysb[:], y_ps[q][:])
                        zrep = work.tile([128, F], BF16, tag='zrep', name='zrep', bufs=2)
                        nc.sync.dma_start(
                            out=zrep[:],
                            in_=self.zs_dram[q * 128:(q + 1) * 128, cols])
                        y2q = work.tile([128, F], BF16, tag=f'y2{q}',
                                        name=f'y2{q}', bufs=2)
                        nc.vector.tensor_mul(y2q[:], ysb[:], zrep[:])
                        y2.append(y2q)
                    for j in range(F // 512):
                        o2 = slice(j * 512, (j + 1) * 512)
                        c2 = slice(ch * F + j * 512, ch * F + (j + 1) * 512)
                        op = ps_a.tile([128, 512], F32, tag=self.mm)
                        nc.tensor.matmul(op[:], w_out[0][:], y2[0][:, o2],
                                         start=True, stop=False)
                        nc.tensor.matmul(op[:], w_out[1][:], y2[1][:, o2],
                                         start=False, stop=True)
                        ot = work.tile([128, 512], F32, tag='ot', name='ot', bufs=2)
                        nc.scalar.copy(ot[:], op[:])
                        nc.sync.dma_start(out=self.out_dram[:, c2], in_=ot[:])

            mt = Mamba('t', xt_s, L, t_out)
            mc = Mamba('c', xg_s, C, c_out)
            # t chunk-0 phase A up front; everything else sprinkled into the
            # V-bound ds loops so Tensor/Scalar fill Vector's gaps
            mt.stage0()
            mt.stageA(0)
            mt.stageB(0, {4: [lambda: mt.stageA(1)], 10: [mt.stageZ]})
            mt.stageB(1, {4: [lambda: mt.stageA(2)], 10: [mc.stageZ]})
            mt.stageB(2, {4: [lambda: mt.stageA(3)], 10: [mc.stage0]})
            mt.stageB(3, {4: [lambda: mc.stageA(0)]})
            mc.stageB(0, {4: [lambda: mc.stageA(1)]})
            mc.stageB(1, {4: [lambda: mc.stageA(2)]})
            mc.stageB(2, {4: [lambda: mc.stageA(3)]})
            mc.stageB(3)

    nc.compile()
    return nc


def _shard_host(inputs):
    """Build per-core input maps from full inputs."""
    x = np.asarray(inputs['x'], np.float32)

    def prep(pfx):
        w_in = np.asarray(inputs[pfx + 'w_in'], np.float32)
        w_xproj = np.asarray(inputs[pfx + 'w_xproj'], np.float32)
        w_dt = np.asarray(inputs[pfx + 'w_dt'], np.float32)
        dteff = w_xproj[:, :DR] @ w_dt
        conv_w = np.asarray(inputs[pfx + 'conv_w'], np.float32).reshape(DC, DI)
        # u-half in-proj scaled per conv tap: col j*DI+di = w_in[d,di]*cw[j,di]
        w_in_u = np.concatenate(
            [w_in[:, :DI] * conv_w[j][None, :] for j in range(DC)], axis=1)
        a_vec = -np.exp(np.asarray(inputs[pfx + 'a_log'], np.float32)[0])
        return {
            pfx + 'w_in_u': w_in_u.astype(BF),
            pfx + 'w_in_z': w_in[:, DI:].astype(BF),
            pfx + 'dteff': dteff.astype(BF),
            pfx + 'w_bc': w_xproj[:, DR:].astype(BF),
            pfx + 'w_out': np.asarray(inputs[pfx + 'w_out'], np.float32).astype(BF),
            pfx + 'conv_b': np.asarray(inputs[pfx + 'conv_b'], np.float32),
            pfx + 'b_dt': np.asarray(inputs[pfx + 'b_dt'], np.float32),
            pfx + 'd': np.asarray(inputs[pfx + 'd'], np.float32),
            pfx + 'a': a_vec,
        }

    shared = {}
    shared.update(prep('t_'))
    shared.update(prep('c_'))
    shared['g_w'] = (np.asarray(inputs['g_w_node'], np.float32) / L).astype(BF)
    shared['g_b'] = np.asarray(inputs['g_b_node'], np.float32)

    in_maps = []
    for k in range(8):
        b, half = k // 2, k % 2
        # temporal tokens: c-major within c-half -> [d, c*L + l]
        xt = x[b, :, 16 * half:16 * (half + 1), :]          # [L, 16, D]
        xt = np.ascontiguousarray(xt.transpose(2, 1, 0).reshape(D, ST))
        # channel tokens own half: l-major -> [d, l*C + c]
        xch = x[b, 128 * half:128 * (half + 1)]             # [128, C, D]
        xch = np.ascontiguousarray(xch.transpose(2, 0, 1).reshape(D, ST))
        xoh = x[b, 128 * (1 - half):128 * (2 - half)]
        xoh = np.ascontiguousarray(xoh.transpose(2, 0, 1).reshape(D, ST))
        m = dict(shared)
        m['xt'] = xt.astype(BF)
        m['xc'] = xch.astype(BF)
        m['xo'] = xoh.astype(BF)
        in_maps.append(m)
    return in_maps


def kernel(**inputs):
    if 'nc' not in _CACHE:
        _CACHE['nc'] = build_program()
    nc = _CACHE['nc']
    in_maps = _shard_host(inputs)
    res = run_bass_kernel_spmd(nc, in_maps, list(range(8)), trace=TRACE)
    _CACHE['last_result'] = res

    T_out = np.zeros((B, L, C, D), np.float32)
    C_out = np.zeros((B, L, C, D), np.float32)
    for k in range(8):
        b, half = k // 2, k % 2
        to = res.results[k]['t_out']          # [d, c*L + l]
        T_out[b, :, 16 * half:16 * (half + 1), :] = \
            to.reshape(D, 16, L).transpose(2, 1, 0)
        co = res.results[k]['c_out']          # [d, l*C + c]
        C_out[b, 128 * half:128 * (half + 1)] = \
            co.reshape(D, 128, C).transpose(1, 2, 0)
    return (T_out, C_out)
